# revision 4
# baseline (speedup 1.0000x reference)
"""Trainium2 Bass kernel for per-class second-moment accumulation.

Computes, for X[B,F], W[F,M], labels[B]:
    feat  = relu(X @ W)                                  [B, M]
    phi   = segment_sum(feat_i feat_i^T by label)        [C, M, M]
    mu    = segment_sum(feat by label)                   [C, M]
    count = per-class counts                             [C]

Strategy: shard by CLASS across the 8 cores (13 class-slots per core).
The host groups sample indices by label into slots of <=128 rows, builds a
per-core gathered X^T with zero padding, and each core computes
relu(Xg @ W) per slot followed by one [128]-contraction matmul group per
slot giving phi_slot = F_s^T F_s (plus a ones-row matmul for mu_slot).
phi rows for each slot stream straight from PSUM -> SBUF -> HBM.
No collectives are needed; the host scatters slot results into the full
[C, M, M] output (adding when a class was split across slots).
"""

import numpy as np

_B, _F, _M, _C = 1024, 768, 512, 100
_NCORES = 8
_SLOTS = 13          # class-slots per core (8*13 = 104 >= C)
_PAD = 128           # padded rows per slot (max samples of one class per slot)
_KT = _F // 128      # 6 contraction tiles for X @ W
_MCH = _M // 128     # 4 output-row chunks per phi slot

_cache: dict = {}


def _build_program():
    import concourse.tile as tile
    from concourse import bacc, mybir

    nc = bacc.Bacc(
        "TRN2", target_bir_lowering=False, debug=False, num_devices=_NCORES
    )

    f32 = mybir.dt.float32
    xgt = nc.dram_tensor("xgt", [_F, _SLOTS * _PAD], f32, kind="ExternalInput")
    w = nc.dram_tensor("w", [_F, _M], f32, kind="ExternalInput")
    phi = nc.dram_tensor("phi", [_SLOTS, _M, _M], f32, kind="ExternalOutput")
    mu = nc.dram_tensor("mu", [_SLOTS, _M], f32, kind="ExternalOutput")

    with tile.TileContext(nc) as tc:
        with (
            tc.tile_pool(name="wts", bufs=1) as wpool,
            tc.tile_pool(name="xg", bufs=1) as xpool,
            tc.tile_pool(name="feat", bufs=1) as fpool,
            tc.tile_pool(name="stage", bufs=6) as spool,
            tc.tile_pool(name="mus", bufs=2) as mspool,
            tc.tile_pool(name="pfeat", bufs=2, space="PSUM") as pfeat,
            tc.tile_pool(name="pphi", bufs=4, space="PSUM") as pphi,
            tc.tile_pool(name="pmu", bufs=2, space="PSUM") as pmu,
        ):
            w_tiles = []
            for k in range(_KT):
                wt = wpool.tile([128, _M], f32, name=f"w{k}")
                nc.sync.dma_start(wt[:], w[k * 128 : (k + 1) * 128, :])
                w_tiles.append(wt)
            ones = wpool.tile([128, 1], f32, name="ones")
            nc.gpsimd.memset(ones[:], 1.0)

            x_tiles = [
                xpool.tile([128, _PAD], f32, name=f"x{k}_{s}")
                for s in range(_SLOTS)
                for k in range(_KT)
            ]
            feat_tiles: list = [None] * _SLOTS
            feat2_tiles: list = [None] * _SLOTS

            def emit_x(s):
                for k in range(_KT):
                    xt = x_tiles[s * _KT + k]
                    nc.sync.dma_start(
                        xt[:],
                        xgt[k * 128 : (k + 1) * 128, s * _PAD : (s + 1) * _PAD],
                    )

            def emit_feat(s):
                ps = pfeat.tile([128, _M], f32, name=f"psf{s}", tag="psf")
                for k in range(_KT):
                    nc.tensor.matmul(
                        ps[:],
                        x_tiles[s * _KT + k][:],
                        w_tiles[k][:],
                        start=(k == 0),
                        stop=(k == _KT - 1),
                    )
                ft = fpool.tile([128, _M], f32, name=f"f{s}")
                nc.scalar.activation(
                    ft[:], ps[:], mybir.ActivationFunctionType.Relu
                )
                feat_tiles[s] = ft
                # HW quirk: a matmul whose lhsT and rhs read the same SBUF
                # tensor wedges the device, so keep a second copy for rhs.
                ft2 = fpool.tile([128, _M], f32, name=f"g{s}")
                nc.vector.tensor_copy(ft2[:], ft[:])
                feat2_tiles[s] = ft2

            def emit_phi_mu(s):
                ft = feat_tiles[s]
                ft2 = feat2_tiles[s]
                for m in range(_MCH):
                    pp = pphi.tile([128, _M], f32, name=f"psp{s}_{m}", tag="psp")
                    nc.tensor.matmul(
                        pp[:],
                        ft[:, m * 128 : (m + 1) * 128],
                        ft2[:],
                        start=True,
                        stop=True,
                    )
                    st = spool.tile([128, _M], f32, name=f"st{s}_{m}", tag="st")
                    if (s * _MCH + m) % 2 == 0:
                        nc.vector.tensor_copy(st[:], pp[:])
                    else:
                        nc.scalar.copy(st[:], pp[:])
                    nc.sync.dma_start(
                        phi[s, m * 128 : (m + 1) * 128, :], st[:]
                    )
                pm = pmu.tile([1, _M], f32, name=f"psm{s}", tag="psm")
                nc.tensor.matmul(pm[:], ones[:], ft[:], start=True, stop=True)
                mt = mspool.tile([1, _M], f32, name=f"mt{s}", tag="mt")
                nc.scalar.copy(mt[:], pm[:])
                nc.sync.dma_start(mu[s : s + 1, :], mt[:])

            emit_x(0)
            emit_x(1)
            emit_feat(0)
            for s in range(_SLOTS):
                if s + 2 < _SLOTS:
                    emit_x(s + 2)
                if s + 1 < _SLOTS:
                    emit_feat(s + 1)
                emit_phi_mu(s)

    nc.compile()
    return nc


def _get_program():
    if "nc" not in _cache:
        _cache["nc"] = _build_program()
    return _cache["nc"]


def _numpy_fallback(X, W, labels):
    feat = np.maximum(X @ W, 0.0).astype(np.float32)
    phi = np.zeros((_C, _M, _M), np.float32)
    mu = np.zeros((_C, _M), np.float32)
    count = np.zeros((_C,), np.float32)
    for c in range(_C):
        idx = np.nonzero(labels == c)[0]
        if len(idx) == 0:
            continue
        fc = feat[idx]
        phi[c] = fc.T @ fc
        mu[c] = fc.sum(axis=0)
        count[c] = len(idx)
    return phi, mu, count


def kernel(X, W, labels):
    X = np.ascontiguousarray(np.asarray(X), dtype=np.float32)
    W = np.ascontiguousarray(np.asarray(W), dtype=np.float32)
    labels = np.asarray(labels).astype(np.int64)
    assert X.shape == (_B, _F) and W.shape == (_F, _M)

    counts = np.bincount(labels, minlength=_C)

    # Group sample indices by class into slots of <= _PAD rows.
    slots: list[tuple[int | None, np.ndarray]] = []
    for c in range(_C):
        idx = np.nonzero(labels == c)[0]
        if len(idx) <= _PAD:
            slots.append((c, idx))
        else:
            for j in range(0, len(idx), _PAD):
                slots.append((c, idx[j : j + _PAD]))
    if len(slots) > _NCORES * _SLOTS:
        return _numpy_fallback(X, W, labels)
    while len(slots) < _NCORES * _SLOTS:
        slots.append((None, np.empty(0, dtype=np.int64)))

    in_maps = []
    for core in range(_NCORES):
        Xg = np.zeros((_SLOTS * _PAD, _F), np.float32)
        for s in range(_SLOTS):
            cls, idx = slots[core * _SLOTS + s]
            if cls is not None and len(idx):
                Xg[s * _PAD : s * _PAD + len(idx)] = X[idx]
        in_maps.append({"xgt": np.ascontiguousarray(Xg.T), "w": W})

    from concourse.bass_utils import run_bass_kernel_spmd

    nc = _get_program()
    res = run_bass_kernel_spmd(nc, in_maps, list(range(_NCORES))).results

    phi = np.zeros((_C, _M, _M), np.float32)
    mu = np.zeros((_C, _M), np.float32)
    for i, (cls, idx) in enumerate(slots):
        if cls is None or len(idx) == 0:
            continue
        core, s = divmod(i, _SLOTS)
        phi[cls] += res[core]["phi"][s]
        mu[cls] += res[core]["mu"][s]
    count = counts.astype(np.float32)
    return phi, mu, count
